# revision 8
# baseline (speedup 1.0000x reference)
"""Contrastive loss kernel for Trainium2 (8 NeuronCores, Bass/Tile).

Math: with L2-normalized embeddings, dist = 1 - sim and MARGIN = 2.0, the
negative branch relu(2 - dist) = 1 + sim is never clipped (|sim| <= 1), so

    pair_loss = (1+sim)^2 - 4*sim*[same]

Summing the strict upper triangle of the symmetric pair matrix:

    total = (B^2 + 2*||s||^2 + ||C||_F^2 - 4*sum_k ||g_k||^2 - sum_i(1-d_i)^2)/2

where C = E^T E (DxD), g_k = sum_{key_i=k} e_i (128 groups), s = sum_i e_i =
column sum of G, d_i = ||e_i||^2. The diagonal term sum_i(1-d_i)^2 is
O(B * 1e-14) and is dropped. This turns an O(B^2 D) problem into O(B D^2).

Sharding: embeddings row-sharded across 8 cores. Each core computes partial
C_p / G_p from its (1024, 256) slab, a 384 KB AllReduce combines them, and
every core finishes the scalar loss on-device.
"""

import sys

for _p in ("/opt/trn_rl_repo",):
    if _p not in sys.path:
        sys.path.insert(0, _p)

import numpy as np

import concourse.bass as bass
import concourse.bacc as bacc
import concourse.mybir as mybir
import concourse.tile as tile
from concourse.bass_utils import run_bass_kernel_spmd

B, D = 8192, 256
N_CORES = 8
ROWS = B // N_CORES          # 1024 rows per core
NT = ROWS // 128             # 8 row-tiles of 128 per core
NKEYS = 128
NUM_PAIRS = B * (B - 1) // 2

F32 = mybir.dt.float32
F32R = mybir.dt.float32r
I32 = mybir.dt.int32

_cache = {}


def _build():
    nc = bacc.Bacc(
        "TRN2",
        target_bir_lowering=False,
        debug=False,
        num_devices=N_CORES,
    )

    emb = nc.dram_tensor("emb", [ROWS, D], F32, kind="ExternalInput").ap()
    # keysT[i, t] = order_keys[slab_start + t*128 + i], as f32 (values < 128 exact)
    keysT = nc.dram_tensor("keysT", [128, NT], F32, kind="ExternalInput").ap()
    loss_out = nc.dram_tensor("loss", [1, 1], F32, kind="ExternalOutput").ap()

    with tile.TileContext(nc) as tc:
        with (
            tc.tile_pool(name="const", bufs=1) as cpool,
            tc.tile_pool(name="work", bufs=3) as pool,
            tc.tile_pool(name="psum", bufs=1, space="PSUM") as psum,
            tc.tile_pool(name="dram", bufs=1, space="DRAM") as dram,
        ):
            keys_sb = cpool.tile([128, NT], F32)
            nc.sync.dma_start(keys_sb[:], keysT[:])

            iota_sb = cpool.tile([128, NKEYS], F32)
            nc.gpsimd.iota(
                iota_sb[:],
                pattern=[[1, NKEYS]],
                base=0,
                channel_multiplier=0,
                allow_small_or_imprecise_dtypes=True,
            )

            ones_sb = cpool.tile([128, 1], F32)
            nc.vector.memset(ones_sb[:], 1.0)

            # Partial Gram C_p (two 128-row halves) and group sums G_p.
            c0 = psum.tile([128, D], F32)
            c1 = psum.tile([128, D], F32)
            g = psum.tile([128, D], F32)

            for t in range(NT):
                et = pool.tile([128, D], F32, tag="emb")
                nc.sync.dma_start(et[:], emb[t * 128 : (t + 1) * 128, :])
                # fp32r matmul inputs must be rounded by their producer
                er = pool.tile([128, D], F32R, tag="embr")
                nc.vector.tensor_copy(er[:], et[:])

                oh = pool.tile([128, NKEYS], F32R, tag="oh")
                nc.vector.tensor_scalar(
                    oh[:],
                    iota_sb[:],
                    keys_sb[:, t : t + 1],
                    None,
                    op0=mybir.AluOpType.is_equal,
                )

                first, last = t == 0, t == NT - 1
                nc.tensor.matmul(
                    c0[:], lhsT=er[:, 0:128], rhs=er[:], start=first, stop=last
                )
                nc.tensor.matmul(
                    c1[:], lhsT=er[:, 128:256], rhs=er[:], start=first, stop=last
                )
                nc.tensor.matmul(
                    g[:], lhsT=oh[:], rhs=er[:], start=first, stop=last
                )

            # Concatenate partials and AllReduce across the 8 cores.
            cat = pool.tile([128, 3 * D], F32)
            nc.vector.tensor_copy(cat[:, 0:D], c0[:])
            nc.vector.tensor_copy(cat[:, D : 2 * D], c1[:])
            nc.vector.tensor_copy(cat[:, 2 * D : 3 * D], g[:])

            cc_in = dram.tile([128, 3 * D], F32)
            cc_out = dram.tile([128, 3 * D], F32)
            nc.sync.dma_start(cc_in[:], cat[:])
            nc.gpsimd.collective_compute(
                "AllReduce",
                mybir.AluOpType.add,
                replica_groups=[list(range(N_CORES))],
                ins=[cc_in.opt()],
                outs=[cc_out.opt()],
            )
            red = pool.tile([128, 3 * D], F32)
            nc.sync.dma_start(red[:], cc_out[:])

            # ||C||^2 and sum_k ||g_k||^2 (free-dim reduce per partition).
            a0 = pool.tile([128, 1], F32)
            a1 = pool.tile([128, 1], F32)
            a2 = pool.tile([128, 1], F32)
            for i, acc in enumerate((a0, a1, a2)):
                sqi = pool.tile([128, D], F32, tag="sq", name=f"sq{i}")
                nc.vector.tensor_mul(sqi[:], red[:, i * D : (i + 1) * D], red[:, i * D : (i + 1) * D])
                nc.vector.tensor_reduce(
                    acc[:], sqi[:], axis=mybir.AxisListType.X, op=mybir.AluOpType.add
                )

            # comb2 = a0 + a1 - 4*a2  per partition
            comb = pool.tile([128, 1], F32)
            nc.vector.tensor_add(comb[:], a0[:], a1[:])
            a2m = pool.tile([128, 1], F32)
            nc.vector.tensor_scalar_mul(a2m[:], a2[:], -4.0)
            comb2 = pool.tile([128, 1], F32)
            nc.vector.tensor_add(comb2[:], comb[:], a2m[:])

            # t1 = sum_p comb2[p]  (cross-partition via ones matmul)
            t1 = psum.tile([1, 1], F32)
            nc.tensor.matmul(t1[:], lhsT=comb2[:], rhs=ones_sb[:], start=True, stop=True)

            # s = column sums of G (cross-partition), then ||s||^2
            srow = psum.tile([1, D], F32)
            nc.tensor.matmul(
                srow[:], lhsT=ones_sb[:], rhs=red[:, 2 * D : 3 * D], start=True, stop=True
            )
            s_sb = pool.tile([1, D], F32)
            nc.vector.tensor_copy(s_sb[:], srow[:])
            s_sq = pool.tile([1, D], F32)
            nc.vector.tensor_mul(s_sq[:], s_sb[:], s_sb[:])
            s2 = pool.tile([1, 1], F32)
            nc.vector.tensor_reduce(
                s2[:], s_sq[:], axis=mybir.AxisListType.X, op=mybir.AluOpType.add
            )

            # loss = (B^2 + 2*s2 + t1) / (2*NUM_PAIRS)
            part = pool.tile([1, 1], F32)
            nc.vector.tensor_scalar_mul(part[:], s2[:], 1.0 / NUM_PAIRS)
            part2 = pool.tile([1, 1], F32)
            nc.vector.tensor_scalar_add(
                part2[:], part[:], float(B) * B / (2.0 * NUM_PAIRS)
            )
            t1m = pool.tile([1, 1], F32)
            nc.vector.tensor_scalar_mul(t1m[:], t1[:], 1.0 / (2.0 * NUM_PAIRS))
            res = pool.tile([1, 1], F32)
            nc.vector.tensor_add(res[:], part2[:], t1m[:])
            nc.sync.dma_start(loss_out[:], res[:])

    nc.compile()
    return nc


def _get_nc():
    if "nc" not in _cache:
        _cache["nc"] = _build()
    return _cache["nc"]


def _in_maps(embeddings: np.ndarray, order_keys: np.ndarray):
    maps = []
    for c in range(N_CORES):
        lo = c * ROWS
        emb_c = np.ascontiguousarray(embeddings[lo : lo + ROWS], dtype=np.float32)
        keys_c = np.ascontiguousarray(
            order_keys[lo : lo + ROWS].astype(np.float32).reshape(NT, 128).T
        )
        maps.append({"emb": emb_c, "keysT": keys_c})
    return maps


def kernel(embeddings: np.ndarray, order_keys: np.ndarray) -> np.ndarray:
    nc = _get_nc()
    res = run_bass_kernel_spmd(nc, _in_maps(embeddings, order_keys), list(range(N_CORES)))
    return np.asarray(res.results[0]["loss"], dtype=np.float32).reshape(())


# revision 9
# speedup vs baseline: 1.4066x; 1.4066x over previous
"""Contrastive loss kernel for Trainium2 (8 NeuronCores, Bass/Tile).

Math: with L2-normalized embeddings, dist = 1 - sim and MARGIN = 2.0, the
negative branch relu(2 - dist) = 1 + sim is never clipped (|sim| <= 1), so

    pair_loss = (1+sim)^2 - 4*sim*[same]

Summing the strict upper triangle of the symmetric pair matrix:

    total = (B^2 + 2*||s||^2 + ||C||_F^2 - 4*sum_k ||g_k||^2)/2

where C = E^T E (DxD), g_k = sum_{key_i=k} e_i (128 key groups), s = sum_i e_i
(= column sum of G). Uses sum_ij sim^2 = tr((E^T E)^2) = ||C||_F^2. The
diagonal correction sum_i(1-||e_i||^2)^2 is O(B*eps^2) ~ 1e-10 and dropped.
This turns an O(B^2 D) problem into O(B D^2).

Distribution: measured on this fabric, an 8-core 384 KB AllReduce costs ~57us
-- far more than the O(B D^2) compute itself. So instead of row-sharding +
AllReduce (the hint), every core redundantly computes the full reduction from
the full embedding matrix (8 MB), which is fully independent per core: no
collective, no cross-core skew sensitivity. Per row-tile of 128 rows, the
concatenation F = [E_tile | onehot(keys_tile)] gives both C and G^T from two
accumulating fp32r matmuls: (F[:, :128])^T F and (F[:, 128:256])^T F.
"""

import sys

for _p in ("/opt/trn_rl_repo",):
    if _p not in sys.path:
        sys.path.insert(0, _p)

import numpy as np

import concourse.bass as bass
import concourse.bacc as bacc
import concourse.mybir as mybir
import concourse.tile as tile
from concourse.bass_utils import run_bass_kernel_spmd

B, D = 8192, 256
N_CORES = 8
NKEYS = 128
NUM_PAIRS = B * (B - 1) // 2
NT = B // 128            # 64 row-tiles of 128 rows
NCHUNK = 8               # DMA granularity: 8 chunks of 8 row-tiles (1 MB each)
TPC = NT // NCHUNK       # row-tiles per chunk
FW = D + NKEYS           # 384: [E | onehot] concat width

F32 = mybir.dt.float32
F32R = mybir.dt.float32r

_cache = {}


def _build():
    nc = bacc.Bacc(
        "TRN2",
        target_bir_lowering=False,
        debug=False,
        num_devices=N_CORES,
    )

    emb = nc.dram_tensor("emb", [B, D], F32, kind="ExternalInput").ap()
    # keysT[i, t] = order_keys[t*128 + i], as f32 (values < 128 exact)
    keysT = nc.dram_tensor("keysT", [128, NT], F32, kind="ExternalInput").ap()
    loss_out = nc.dram_tensor("loss", [1, 1], F32, kind="ExternalOutput").ap()

    # emb rows (c*TPC*128 + t*128 + p) viewed as [c][p][t][d] for chunked DMA
    emb_v = emb.rearrange("(c t p) d -> c p t d", c=NCHUNK, t=TPC, p=128)

    with tile.TileContext(nc) as tc:
        with (
            tc.tile_pool(name="const", bufs=1) as cpool,
            tc.tile_pool(name="work", bufs=3) as pool,
            tc.tile_pool(name="psum", bufs=1, space="PSUM") as psum,
        ):
            keys_sb = cpool.tile([128, NT], F32)
            nc.sync.dma_start(keys_sb[:], keysT[:])

            iota_sb = cpool.tile([128, NKEYS], F32)
            nc.gpsimd.iota(
                iota_sb[:],
                pattern=[[1, NKEYS]],
                base=0,
                channel_multiplier=0,
                allow_small_or_imprecise_dtypes=True,
            )

            ones_sb = cpool.tile([128, 1], F32)
            nc.vector.memset(ones_sb[:], 1.0)

            # p0 = [C[0:128,:] | G^T[0:128,:]], p1 = [C[128:256,:] | G^T[128:256,:]]
            p0 = psum.tile([128, FW], F32, name="p0")
            p1 = psum.tile([128, FW], F32, name="p1")

            for c in range(NCHUNK):
                ech = pool.tile([128, TPC, D], F32, tag="ech", bufs=2)
                nc.sync.dma_start(ech[:], emb_v[c])
                for t in range(TPC):
                    gi = c * TPC + t
                    ft = pool.tile([128, FW], F32R, tag="ft", bufs=4)
                    nc.vector.tensor_copy(ft[:, 0:D], ech[:, t, :])
                    nc.vector.tensor_scalar(
                        ft[:, D:FW],
                        iota_sb[:],
                        keys_sb[:, gi : gi + 1],
                        None,
                        op0=mybir.AluOpType.is_equal,
                    )
                    first, last = gi == 0, gi == NT - 1
                    nc.tensor.matmul(
                        p0[:], lhsT=ft[:, 0:128], rhs=ft[:], start=first, stop=last
                    )
                    nc.tensor.matmul(
                        p1[:], lhsT=ft[:, 128:256], rhs=ft[:], start=first, stop=last
                    )

            # Move PSUM partials to SBUF for multi-read finals.
            r0 = pool.tile([128, FW], F32)
            nc.vector.tensor_copy(r0[:], p0[:])
            r1 = pool.tile([128, FW], F32)
            nc.vector.tensor_copy(r1[:], p1[:])

            # Per-partition (= per C-row / per embedding-dim a) pieces:
            #   aC = sum_col C[a,:]^2, aG = sum_k G^T[a,k]^2, s_a = sum_k G^T[a,k]
            aC0 = pool.tile([128, 1], F32)
            aC1 = pool.tile([128, 1], F32)
            aG0 = pool.tile([128, 1], F32)
            aG1 = pool.tile([128, 1], F32)
            s0 = pool.tile([128, 1], F32)
            s1 = pool.tile([128, 1], F32)
            for r, aC, aG, s in ((r0, aC0, aG0, s0), (r1, aC1, aG1, s1)):
                sqC = pool.tile([128, D], F32, tag="sqC", name=f"sqC_{aC.name}")
                nc.vector.tensor_mul(sqC[:], r[:, 0:D], r[:, 0:D])
                nc.vector.tensor_reduce(
                    aC[:], sqC[:], axis=mybir.AxisListType.X, op=mybir.AluOpType.add
                )
                sqG = pool.tile([128, NKEYS], F32, tag="sqG", name=f"sqG_{aG.name}")
                nc.vector.tensor_mul(sqG[:], r[:, D:FW], r[:, D:FW])
                nc.vector.tensor_reduce(
                    aG[:], sqG[:], axis=mybir.AxisListType.X, op=mybir.AluOpType.add
                )
                nc.vector.tensor_reduce(
                    s[:], r[:, D:FW], axis=mybir.AxisListType.X, op=mybir.AluOpType.add
                )

            # comb = (aC0+aC1) - 4*(aG0+aG1) + 2*(s0^2+s1^2)   per partition
            tC = pool.tile([128, 1], F32)
            nc.vector.tensor_add(tC[:], aC0[:], aC1[:])
            tG = pool.tile([128, 1], F32)
            nc.vector.tensor_add(tG[:], aG0[:], aG1[:])
            tGm = pool.tile([128, 1], F32)
            nc.vector.tensor_scalar_mul(tGm[:], tG[:], -4.0)
            ssq0 = pool.tile([128, 1], F32)
            nc.vector.tensor_mul(ssq0[:], s0[:], s0[:])
            ssq1 = pool.tile([128, 1], F32)
            nc.vector.tensor_mul(ssq1[:], s1[:], s1[:])
            tS = pool.tile([128, 1], F32)
            nc.vector.tensor_add(tS[:], ssq0[:], ssq1[:])
            tSm = pool.tile([128, 1], F32)
            nc.vector.tensor_scalar_mul(tSm[:], tS[:], 2.0)
            comb = pool.tile([128, 1], F32)
            nc.vector.tensor_add(comb[:], tC[:], tGm[:])
            comb2 = pool.tile([128, 1], F32)
            nc.vector.tensor_add(comb2[:], comb[:], tSm[:])

            # t1 = sum_p comb2[p] via ones matmul, then affine to the loss.
            t1 = psum.tile([1, 1], F32, name="t1")
            nc.tensor.matmul(t1[:], lhsT=comb2[:], rhs=ones_sb[:], start=True, stop=True)
            t1m = pool.tile([1, 1], F32)
            nc.vector.tensor_scalar_mul(t1m[:], t1[:], 1.0 / (2.0 * NUM_PAIRS))
            res = pool.tile([1, 1], F32)
            nc.vector.tensor_scalar_add(
                res[:], t1m[:], float(B) * B / (2.0 * NUM_PAIRS)
            )
            nc.sync.dma_start(loss_out[:], res[:])

    nc.compile()
    return nc


def _get_nc():
    if "nc" not in _cache:
        _cache["nc"] = _build()
    return _cache["nc"]


def _in_maps(embeddings: np.ndarray, order_keys: np.ndarray):
    emb = np.ascontiguousarray(embeddings, dtype=np.float32)
    keys = np.ascontiguousarray(
        order_keys.astype(np.float32).reshape(NT, 128).T
    )
    return [{"emb": emb, "keysT": keys} for _ in range(N_CORES)]


def kernel(embeddings: np.ndarray, order_keys: np.ndarray) -> np.ndarray:
    nc = _get_nc()
    res = run_bass_kernel_spmd(nc, _in_maps(embeddings, order_keys), list(range(N_CORES)))
    return np.asarray(res.results[0]["loss"], dtype=np.float32).reshape(())
